# revision 9
# baseline (speedup 1.0000x reference)
"""Trainium2 Bass kernel v2 for nn_MDFO (CNL + PNL non-local blocks + CBAM + fusion).

Restructured from baseline for latency:
  - x / x0 / out shipped bf16 (halves input+output DMA, kills on-device casts)
  - consts packed into 4 blob DMAs
  - S-blocks as packed [128,128] matmuls interleaved into stage A
  - theta bias folded via an extra ones-column in the att contraction, so
    thT PSUM->SBUF copies are plain and can run on the ACT engine
  - Pool engine recruited; pointwise split DVE/Pool/ACT
  - running bf16 max-accumulate stats during zp phase (2x DVE mode)
  - tail: SBUF->SBUF reshape DMAs (no DRAM roundtrips), natural-order Sdy,
    early act-table warmup, fixed-assignment DVE->Pool final pipeline
"""
import sys

import numpy as np

sys.path.insert(0, "/opt/trn_rl_repo")

import ml_dtypes  # noqa: E402

import concourse.bass as bass  # noqa: E402
import concourse.bacc as bacc  # noqa: E402
import concourse.tile as tile  # noqa: E402
from concourse import mybir  # noqa: E402
from concourse.bass_utils import run_bass_kernel_spmd  # noqa: E402

EPS = 1e-5
F32 = mybir.dt.float32
F32R = mybir.dt.float32r
BF16 = mybir.dt.bfloat16
AF = mybir.ActivationFunctionType
ALU = mybir.AluOpType

Ch, Cl, H, W = 256, 128, 64, 64
N = H * W            # 4096
M = N // 2           # 2048


def _R(ap):
    return ap.bitcast(F32R)


# ---------------- const blob layouts (column offsets) ----------------
BF_E = {'w_x0cat': (0, 256), 'w_th': (256, 512), 'w_g': (512, 640),
        'w_th2': (640, 768), 'ones1b': (768, 896), 'b_x0row': (896, 1152)}
BF_E_COLS = 1152
BF_L = {'identb': (0, 128), 'Kcat2': (128, 1024), 'E_shift': (1024, 1088),
        'Sdy2b': (1088, 1984)}
BF_L_COLS = 1984
F32A = {'b_x0cat': (0, 256), 'b_th': (256, 384), 'b_g': (384, 385)}
F32A_COLS = 392
F32L = {
    'w_cnlW': (0, 256),          # [128, 256]
    'w_tyT': (256, 320),         # [128, 64]
    'w_pnlW': (320, 576),        # rows 0:64
    'b_th2': (576, 577),         # rows 0:64
    'b2': (577, 579),            # [128, 2]
    'fc1T': (579, 611),          # [128, 32]
    'fc2T': (611, 867),          # rows 0:16
    'ones1': (867, 995),         # row 0 only
    'ident64': (995, 1059),      # rows 0:64
    'Sdy2': (1059, 1955),        # rows 0:32 [32, 896]
}
F32L_COLS = 1960


def fold_params(inp):
    """Host-side constant folding -> blobs."""
    f = {}
    scale1 = inp['cnl_bn_g'] / np.sqrt(inp['cnl_bn_v'] + EPS)
    cnl_bf = (inp['cnl_W_b'] * scale1 + inp['cnl_bn_b']
              - inp['cnl_bn_m'] * scale1).astype(np.float32)
    scale2 = inp['pnl_bn_g'] / np.sqrt(inp['pnl_bn_v'] + EPS)
    pnl_bf = (inp['pnl_W_b'] * scale2 + inp['pnl_bn_b']
              - inp['pnl_bn_m'] * scale2).astype(np.float32)
    w_fuse = float(inp['fusion_weight'])
    f['w_fuse'] = w_fuse

    bfE = np.zeros((128, BF_E_COLS), dtype=np.float32)

    def put(blob, layout, key, arr):
        lo, hi = layout[key]
        r, c = arr.shape
        assert c == hi - lo, (key, c, hi - lo)
        blob[:r, lo:hi] = arr

    put(bfE, BF_E, 'w_x0cat', np.concatenate([
        inp['cnl_phi_w'].T, inp['pnl_phi_w'].T, (inp['pnl_g_w'] / M).T,
    ], axis=1).astype(np.float32))
    thT = inp['cnl_theta_w'].T.astype(np.float32)                   # [256, 128]
    put(bfE, BF_E, 'w_th', np.concatenate([thT[:128], thT[128:]], axis=1))
    put(bfE, BF_E, 'w_g', (inp['cnl_g_w'] / Cl).T.astype(np.float32))
    th2 = inp['pnl_theta_w'].T.astype(np.float32)                   # [256, 64]
    put(bfE, BF_E, 'w_th2', np.concatenate([th2[:128], th2[128:]], axis=1))
    put(bfE, BF_E, 'ones1b', np.ones((1, 128), dtype=np.float32))

    bfL = np.zeros((128, BF_L_COLS), dtype=np.float32)
    put(bfL, BF_L, 'identb', np.eye(128, dtype=np.float32))
    Esh = np.zeros((128, 64), dtype=np.float32)
    Esh[64:128] = np.eye(64, dtype=np.float32)
    put(bfL, BF_L, 'E_shift', Esh)

    f32A = np.zeros((128, F32A_COLS), dtype=np.float32)
    brow = np.concatenate([
        inp['cnl_phi_b'], inp['pnl_phi_b'], inp['pnl_g_b'] / M]).astype(np.float32)
    put(bfE, BF_E, 'b_x0row', brow[None, :])
    f['bfE'] = bfE.astype(ml_dtypes.bfloat16)
    put(f32A, F32A, 'b_th', np.tile(inp['cnl_theta_b'][None, :], (128, 1)))
    put(f32A, F32A, 'b_g', (inp['cnl_g_b'] / Cl).astype(np.float32)[:, None])
    f['f32A'] = f32A

    f32L = np.zeros((128, F32L_COLS), dtype=np.float32)
    Wf = (scale1[:, None] * inp['cnl_W_w']).astype(np.float32)      # [256, 128]
    put(f32L, F32L, 'w_cnlW', Wf.T)
    put(f32L, F32L, 'w_tyT', (inp['pnl_theta_w'] @ Wf).T)
    put(f32L, F32L, 'w_pnlW', (scale2[:, None] * inp['pnl_W_w']).T)
    b_th2 = (inp['pnl_theta_b'] + inp['pnl_theta_w'] @ cnl_bf).astype(np.float32)
    put(f32L, F32L, 'b_th2', b_th2[:, None])
    bias2 = (pnl_bf + cnl_bf).astype(np.float32)
    put(f32L, F32L, 'b2', np.stack([bias2[:128], bias2[128:]], axis=1))
    fc1 = inp['ca_fc1_w'].T.astype(np.float32)                      # [256, 16]
    put(f32L, F32L, 'fc1T', np.concatenate([fc1[:128], fc1[128:]], axis=1))
    put(f32L, F32L, 'fc2T', inp['ca_fc2_w'].T.astype(np.float32))
    put(f32L, F32L, 'ones1', np.ones((128, 128), dtype=np.float32))
    put(f32L, F32L, 'ident64', np.eye(64, dtype=np.float32))

    # sa conv: banded x-conv mats; 1/256 and 1/w_fuse folds
    sa_w = np.asarray(inp['sa_conv_w'][0], dtype=np.float32).copy()  # [2, 7, 7]
    sa_w[0] /= (256.0 * w_fuse)
    sa_w[1] /= w_fuse
    Kcat = np.zeros((2, 64, 7 * 64), dtype=np.float32)
    for ch in range(2):
        for dy in range(7):
            for dx in range(7):
                w_ = sa_w[ch, dy, dx]
                if w_ == 0.0:
                    continue
                for x in range(64):
                    xp = x + dx - 3
                    if 0 <= xp < 64:
                        Kcat[ch, xp, dy * 64 + x] = w_
    kc2 = np.concatenate([Kcat[0], Kcat[1]], axis=1)
    bfL[0:64, BF_L['Kcat2'][0]:BF_L['Kcat2'][1]] = kc2
    bfL[64:128, BF_L['Kcat2'][0]:BF_L['Kcat2'][1]] = kc2
    f['bfL'] = bfL.astype(ml_dtypes.bfloat16)
    # Sdy2 [32, 2, 448] -> [32, 896]: Sdy2[g, par, dy*64+y]=1 iff y+dy-3 == 2g+par
    Sdy2 = np.zeros((32, 2, 7 * 64), dtype=np.float32)
    for g in range(32):
        for par in range(2):
            for dy in range(7):
                y = 2 * g + par - dy + 3
                if 0 <= y < 64:
                    Sdy2[g, par, dy * 64 + y] = 1.0
    put(f32L, F32L, 'Sdy2', Sdy2.reshape(32, 896))
    bfL2 = np.zeros((128, 896), dtype=np.float32)
    bfL2[:32] = Sdy2.reshape(32, 896)
    bfL[:, BF_L['Sdy2b'][0]:BF_L['Sdy2b'][1]] = bfL2
    f['bfL'] = bfL.astype(ml_dtypes.bfloat16)
    f['f32L'] = f32L
    return f


def build_nc(w_fuse):
    nc = bacc.Bacc(None)
    x_d = nc.declare_dram_parameter("x", [128, 2 * N], BF16, isOutput=False)
    x0_d = nc.declare_dram_parameter("x0", [128, N], BF16, isOutput=False)
    bfE_d = nc.declare_dram_parameter("bfE", [128, BF_E_COLS], BF16, isOutput=False)
    bfL_d = nc.declare_dram_parameter("bfL", [128, BF_L_COLS], BF16, isOutput=False)
    f32A_d = nc.declare_dram_parameter("f32A", [128, F32A_COLS], F32, isOutput=False)
    f32L_d = nc.declare_dram_parameter("f32L", [128, F32L_COLS], F32R, isOutput=False)
    out_d = nc.declare_dram_parameter("out", [256, N], BF16, isOutput=True)
    smean_d = nc.dram_tensor("smean", [1, N], F32R)
    ssig_d = nc.dram_tensor("ssig", [1, N], F32R)

    with tile.TileContext(nc) as tc:
        _frees = []

        def _keep(pair):
            _frees.append(pair[1])
            return pair[0]

        # ---- persistent SBUF tensors ----
        x_t = _keep(tc.tile([128, 2, N], BF16, name="x_t"))
        x0_t = _keep(tc.tile([128, N], BF16, name="x0_t"))
        bfE = _keep(tc.tile([128, BF_E_COLS], BF16, name="bfE"))
        bfL = _keep(tc.tile([128, BF_L_COLS], BF16, name="bfL"))
        f32A = _keep(tc.tile([128, F32A_COLS], F32, name="f32A"))
        f32L = _keep(tc.tile([128, F32L_COLS], F32R, name="f32L"))
        x0cat = _keep(tc.tile([128, 32, 256], BF16, name="x0cat"))
        thT = _keep(tc.tile([128, 32, 132], BF16, name="thT"))
        # (col 128 of each thT slot holds 1.0 for the phi-colsum att column)
        gxy = _keep(tc.tile([128, N], F32R, name="gxy"))     # g_x, then y in-place
        attT = _keep(tc.tile([128, 128], F32R, name="attT"))
        S2 = _keep(tc.tile([64, 256], BF16, name="S2"))
        T_t = _keep(tc.tile([64, N], BF16, name="T_t"))
        Y_t = _keep(tc.tile([64, N], F32R, name="Y_t"))
        z_t = _keep(tc.tile([128, 2, N], BF16, name="z_t"))
        macc = _keep(tc.tile([128, 2, 512], BF16, name="macc"))
        psum_cols = _keep(tc.tile([128, 2, 8], F32, name="psum_cols"))
        V_t = _keep(tc.tile([128, 2, 2], F32R, name="V_t"))
        h_t = _keep(tc.tile([16, 2], F32R, name="h_t"))
        ca_t = _keep(tc.tile([128, 2], F32, name="ca_t"))
        ca_bf = _keep(tc.tile([128, 2], BF16, name="ca_bf"))
        tmp1 = _keep(tc.tile([128, 4], F32, name="tmp1"))
        warm = _keep(tc.tile([1, 2], F32, name="warm"))
        xh = _keep(tc.tile([128, 2, N], BF16, name="xh"))
        U_t = _keep(tc.tile([128, 2, N], BF16, name="U_t"))
        tA = _keep(tc.tile([128, N], BF16, name="tA"))
        PM = _keep(tc.tile([128, 32], BF16, name="PM"))
        PMo = _keep(tc.tile([64, 32], BF16, name="PMo"))
        meanrow = _keep(tc.tile([1, N], F32R, name="meanrow"))
        mean2d = _keep(tc.tile([64, 64], F32R, name="mean2d"))
        mapT2 = _keep(tc.tile([64, 2, 32], BF16, name="mapT2"))
        R_sb = _keep(tc.tile([32, 2, 448], BF16, name="R_sb"))
        sig2d = _keep(tc.tile([64, 64], F32R, name="sig2d"))
        sigrow = _keep(tc.tile([1, N], F32R, name="sigrow"))

        from contextlib import ExitStack
        stack = ExitStack()

        def C(name):
            if name in BF_E:
                lo, hi = BF_E[name]
                return bfE[:, lo:hi]
            if name in BF_L:
                lo, hi = BF_L[name]
                return bfL[:, lo:hi]
            if name in F32A:
                lo, hi = F32A[name]
                return f32A[:, lo:hi]
            lo, hi = F32L[name]
            return f32L[:, lo:hi]

        # ---- DMA order: early consts, x0/x quarters, late consts ----
        nc.sync.dma_start(out=x0_t[:, 0:1024], in_=x0_d[:, 0:1024])
        nc.sync.dma_start(out=bfE[:, :], in_=bfE_d[:, :])
        nc.sync.dma_start(out=x_t[:, 0, 0:1024], in_=x_d[:, 0:1024])
        nc.sync.dma_start(out=x_t[:, 1, 0:1024], in_=x_d[:, N:N + 1024])
        nc.sync.dma_start(out=f32A[:, :], in_=f32A_d[:, :])
        for q in range(1, 4):
            nc.sync.dma_start(out=x0_t[:, bass.ts(q, 1024)],
                              in_=x0_d[:, bass.ts(q, 1024)])
            nc.sync.dma_start(out=x_t[:, 0, bass.ts(q, 1024)],
                              in_=x_d[:, q * 1024:(q + 1) * 1024])
            nc.sync.dma_start(out=x_t[:, 1, bass.ts(q, 1024)],
                              in_=x_d[:, N + q * 1024:N + (q + 1) * 1024])
        nc.sync.dma_start(out=f32L[:, :], in_=f32L_d[:, :])
        nc.sync.dma_start(out=bfL[:, :], in_=bfL_d[:, :])

        ps = stack.enter_context(tc.tile_pool(name="ps", bufs=4, space="PSUM"))
        tx = stack.enter_context(tc.tile_pool(name="tx", bufs=2, space="PSUM"))
        acc = stack.enter_context(tc.tile_pool(name="acc", bufs=1, space="PSUM"))
        sp = stack.enter_context(tc.tile_pool(name="sp", bufs=3))



        nc.scalar.activation(out=warm[:, 0:1], in_=bfE[0:1, 0:1], func=AF.Sigmoid)
        nc.scalar.activation(out=warm[:, 1:2], in_=bfE[0:1, 0:1], func=AF.Relu)
        nc.vector.memset(thT[:, :, 128:129], 1.0)

        # =========== Stage A sweep 1: conv matmuls + copies ===========
        att_ps = acc.tile([128, 132], F32, tag="att", name="att_ps")
        S_ps = acc.tile([64, 256], F32, tag="S", name="S_ps")
        for i in range(32):
            # x0cat conv [128 n, 256]: rank-1 bias matmul + conv, plain copy out
            ps_a = ps.tile([128, 256], F32, tag="ps", name="ps_a")
            nc.tensor.matmul(ps_a[:, :], C('ones1b')[0:1, :], C('b_x0row')[0:1, :],
                             start=True, stop=False)
            nc.tensor.matmul(ps_a[:, :], x0_t[:, bass.ts(i, 128)],
                             C('w_x0cat'), start=False, stop=True)
            if i % 3 == 0:
                nc.vector.tensor_copy(out=x0cat[:, i, :], in_=ps_a[:, :])
            else:
                nc.scalar.copy(out=x0cat[:, i, :], in_=ps_a[:, :])
            # thT (no bias; folded into att via ones col)
            ps_b = tx.tile([128, 128], F32, tag="tx", name="ps_b")
            nc.tensor.matmul(ps_b[:, :], x_t[:, 0, bass.ts(i, 128)],
                             C('w_th')[:, 0:128], start=True, stop=False)
            nc.tensor.matmul(ps_b[:, :], x_t[:, 1, bass.ts(i, 128)],
                             C('w_th')[:, 128:256], start=False, stop=True)
            if i % 2 == 0:
                nc.scalar.copy(out=thT[:, i, 0:128], in_=ps_b[:, :])
            else:
                nc.vector.tensor_copy(out=thT[:, i, 0:128], in_=ps_b[:, :])
            # g_x per 512-col tile
            if i % 4 == 3:
                t = i // 4
                ps_g = ps.tile([128, 512], F32, tag="ps", name="ps_g")
                nc.tensor.matmul(ps_g[:, :], C('w_g'),
                                 x0_t[:, bass.ts(t, 512)], start=True, stop=True)
                if t % 2 == 0:
                    nc.vector.tensor_scalar(out=gxy[:, bass.ts(t, 512)],
                                            in0=ps_g[:, :], scalar1=C('b_g'),
                                            scalar2=None, op0=ALU.add)
                else:
                    nc.scalar.activation(out=gxy[:, bass.ts(t, 512)],
                                         in_=ps_g[:, :], func=AF.Identity,
                                         bias=C('b_g'))
                if t % 2 == 1:
                    q = t // 2
                    for ch in range(2):
                        nc.vector.tensor_scalar(
                            out=xh[:, ch, bass.ts(q, 1024)],
                            in0=x_t[:, ch, bass.ts(q, 1024)],
                            scalar1=1.0 - w_fuse, scalar2=None, op0=ALU.mult)
        # =========== Stage A sweep 2: att + S contractions ===========
        for i in range(32):
            nc.tensor.matmul(att_ps[:, 0:129], x0cat[:, i, 0:128],
                             thT[:, i, 0:129], start=(i == 0), stop=(i == 31))
        for blk, (pa, pb) in enumerate(((0, 0), (0, 16), (16, 0), (16, 16))):
            for j in range(16):
                nc.tensor.matmul(
                    S_ps[:, bass.ts(blk, 64)],
                    x0cat[:, j + pa, 128:192],
                    x0cat[:, j + pb, 192:256],
                    start=(j == 0), stop=(j == 15))
        nc.vector.scalar_tensor_tensor(
            out=attT[:, :], in0=C('b_th'), scalar=att_ps[:, 128:129],
            in1=att_ps[:, 0:128], op0=ALU.mult, op1=ALU.add)
        nc.scalar.copy(out=S2[:, :], in_=S_ps[:, :])

        # =========== y = attT^T-contract g_x (in-place into gxy) ===========
        for t in range(8):
            ps_y = ps.tile([128, 512], F32, tag="ps", name="ps_y")
            nc.tensor.matmul(ps_y[:, :], _R(attT[:, :]),
                             _R(gxy[:, bass.ts(t, 512)]), start=True, stop=True)
            nc.vector.tensor_copy(out=gxy[:, bass.ts(t, 512)], in_=ps_y[:, :])

        # =========== T = w_tyT.y + w_th2.x + b_th2  [64, N] bf16 ===========
        for t in range(8):
            ps_t = ps.tile([64, 512], F32, tag="ps", name="ps_t")
            nc.tensor.matmul(ps_t[:, :], _R(C('w_tyT')),
                             _R(gxy[:, bass.ts(t, 512)]), start=True, stop=False)
            nc.tensor.matmul(ps_t[:, :], C('w_th2')[:, 0:64],
                             x_t[:, 0, bass.ts(t, 512)], start=False, stop=False)
            nc.tensor.matmul(ps_t[:, :], C('w_th2')[:, 64:128],
                             x_t[:, 1, bass.ts(t, 512)], start=False, stop=True)
            if t % 2 == 0:
                nc.scalar.activation(out=T_t[:, bass.ts(t, 512)], in_=ps_t[:, :],
                                     func=AF.Identity,
                                     bias=C('b_th2')[0:64, :].bitcast(F32))
            else:
                nc.vector.tensor_scalar(out=T_t[:, bass.ts(t, 512)],
                                        in0=ps_t[:, :],
                                        scalar1=C('b_th2')[0:64, :].bitcast(F32),
                                        scalar2=None, op0=ALU.add)

        # =========== Y [64, N] f32 ===========
        for h in range(2):
            for t in range(4):
                ps_Y = ps.tile([64, 512], F32, tag="ps", name="ps_Y")
                nc.tensor.matmul(ps_Y[:, :], S2[:, h * 64:h * 64 + 64],
                                 T_t[:, bass.ts(t, 512)], start=True, stop=False)
                nc.tensor.matmul(ps_Y[:, :], S2[:, 128 + h * 64:192 + h * 64],
                                 T_t[:, M + t * 512:M + (t + 1) * 512],
                                 start=False, stop=True)
                if t % 2 == 0:
                    nc.vector.tensor_copy(
                        out=Y_t[:, h * M + t * 512:h * M + (t + 1) * 512],
                        in_=ps_Y[:, :])
                else:
                    nc.scalar.copy(
                        out=Y_t[:, h * M + t * 512:h * M + (t + 1) * 512],
                        in_=ps_Y[:, :])

        # =========== z_pnl (bf16) + running stats ===========
        for t in range(8):
            for ch in range(2):
                ps_p = ps.tile([128, 512], F32, tag="ps", name="ps_p")
                nc.tensor.matmul(ps_p[:, :], _R(C('w_pnlW')[0:64, bass.ts(ch, 128)]),
                                 _R(Y_t[:, bass.ts(t, 512)]), start=True, stop=False)
                nc.tensor.matmul(ps_p[:, :], _R(C('w_cnlW')[:, bass.ts(ch, 128)]),
                                 _R(gxy[:, bass.ts(t, 512)]), start=False, stop=True)
                nc.vector.scalar_tensor_tensor(
                    out=z_t[:, ch, bass.ts(t, 512)], in0=ps_p[:, :],
                    scalar=C('b2')[:, ch:ch + 1].bitcast(F32),
                    in1=x_t[:, ch, bass.ts(t, 512)], op0=ALU.add, op1=ALU.add,
                    accum_out=psum_cols[:, ch, t:t + 1])
                if ch == 1:
                    if t == 0:
                        nc.vector.tensor_scalar(
                            out=macc[:, :, :], in0=z_t[:, :, 0:512],
                            scalar1=1.0, scalar2=None, op0=ALU.mult)
                    else:
                        nc.vector.tensor_tensor(
                            out=macc[:, :, :], in0=macc[:, :, :],
                            in1=z_t[:, :, bass.ts(t, 512)], op=ALU.max)

        # =========== CBAM channel attention ===========
        nc.vector.reduce_max(out=tmp1[:, 0:2], in_=macc[:, :, :],
                             axis=mybir.AxisListType.X)
        nc.vector.reduce_sum(out=tmp1[:, 2:4], in_=psum_cols[:, :, :],
                             axis=mybir.AxisListType.X)
        for ch in range(2):
            nc.scalar.activation(out=V_t[:, ch, 0:1], in_=tmp1[:, 2 + ch:3 + ch],
                                 func=AF.Identity, scale=1.0 / float(N))
            nc.scalar.activation(out=V_t[:, ch, 1:2], in_=tmp1[:, ch:ch + 1],
                                 func=AF.Identity)
        ps_f1 = ps.tile([16, 2], F32, tag="ps", name="ps_f1")
        nc.tensor.matmul(ps_f1[:, :], _R(C('fc1T')[:, 0:16]), _R(V_t[:, 0, :]),
                         start=True, stop=False)
        nc.tensor.matmul(ps_f1[:, :], _R(C('fc1T')[:, 16:32]), _R(V_t[:, 1, :]),
                         start=False, stop=True)
        nc.scalar.activation(out=h_t[:, :], in_=ps_f1[:, :], func=AF.Relu)
        for ch in range(2):
            ps_f2 = ps.tile([128, 2], F32, tag="ps", name="ps_f2")
            nc.tensor.matmul(ps_f2[:, :], _R(C('fc2T')[0:16, bass.ts(ch, 128)]),
                             _R(h_t[:, :]), start=True, stop=True)
            nc.vector.tensor_copy(out=tmp1[:, 2:4], in_=ps_f2[:, :])
            nc.vector.tensor_tensor(out=tmp1[:, 0:1], in0=tmp1[:, 2:3],
                                    in1=tmp1[:, 3:4], op=ALU.add)
            nc.scalar.activation(out=ca_t[:, ch:ch + 1], in_=tmp1[:, 0:1],
                                 func=AF.Sigmoid)
        nc.vector.tensor_scalar_mul(ca_t[:, :], ca_t[:, :], w_fuse)
        nc.vector.tensor_copy(out=ca_bf[:, :], in_=ca_t[:, :])

        # =========== max map + mean map interleaved (also builds U) ===========
        for q in range(4):
            nc.vector.tensor_scalar(out=U_t[:, 0, bass.ts(q, 1024)],
                                    in0=z_t[:, 0, bass.ts(q, 1024)],
                                    scalar1=ca_t[:, 0:1], scalar2=None,
                                    op0=ALU.mult)
            nc.vector.tensor_scalar(out=U_t[:, 1, bass.ts(q, 1024)],
                                    in0=z_t[:, 1, bass.ts(q, 1024)],
                                    scalar1=ca_t[:, 1:2], scalar2=None,
                                    op0=ALU.mult)
            nc.vector.tensor_tensor(out=tA[:, bass.ts(q, 1024)],
                                    in0=U_t[:, 0, bass.ts(q, 1024)],
                                    in1=U_t[:, 1, bass.ts(q, 1024)], op=ALU.max)
            for b4 in range(2):
                ps_tx = tx.tile([128, 4, 128], BF16, tag="tx", name="ps_tx")
                for k in range(4):
                    g = 8 * q + 4 * b4 + k
                    nc.tensor.transpose(ps_tx[:, k, :], tA[:, bass.ts(g, 128)],
                                        C('identb'))
                nc.vector.reduce_max(out=PM[:, bass.ts(2 * q + b4, 4)],
                                     in_=ps_tx[:, :, :], axis=mybir.AxisListType.X)
                t = 2 * q + b4
                ps_m = ps.tile([1, 512], F32, tag="ps", name="ps_m")
                nc.tensor.matmul(ps_m[:, :], ca_bf[:, 0:1],
                                 z_t[:, 0, bass.ts(t, 512)], start=True, stop=False)
                nc.tensor.matmul(ps_m[:, :], ca_bf[:, 1:2],
                                 z_t[:, 1, bass.ts(t, 512)], start=False, stop=True)
                nc.scalar.copy(out=meanrow[:, bass.ts(t, 512)], in_=ps_m[:, :])
        nc.sync.dma_start(out=mean2d[:, :],
                          in_=meanrow[:, :].rearrange("p (y x) -> p y x", y=64))
        ps_tm = tx.tile([64, 64], F32R, tag="tx", name="ps_tm")
        nc.tensor.transpose(ps_tm[:, :], _R(mean2d[:, :]),
                            _R(C('ident64')[0:64, :]))
        nc.vector.tensor_copy(
            out=mapT2[:, :, :],
            in_=ps_tm[:, :].bitcast(F32).rearrange("p (c two) -> p two c", two=2))

        # =========== sa conv (split even/odd y) ===========
        Kc = C('Kcat2')
        Sdy = C('Sdy2b')
        ps_pmo = ps.tile([64, 32], F32, tag="ps", name="ps_pmo")
        nc.tensor.matmul(ps_pmo[:, :], C('E_shift'), PM[:, :], start=True, stop=True)
        nc.vector.tensor_copy(out=PMo[:, :], in_=ps_pmo[:, :])
        ps_Re = ps.tile([32, 448], F32, tag="ps", name="ps_Re")
        nc.tensor.matmul(ps_Re[:, :], mapT2[:, 0, :], Kc[0:64, 0:448],
                         start=True, stop=False)
        nc.tensor.matmul(ps_Re[:, :], PM[0:64, :], Kc[0:64, 448:896],
                         start=False, stop=True)
        nc.vector.tensor_copy(out=R_sb[:, 0, :], in_=ps_Re[:, :])
        ps_Ro = ps.tile([32, 448], F32, tag="ps", name="ps_Ro")
        nc.tensor.matmul(ps_Ro[:, :], mapT2[:, 1, :], Kc[0:64, 0:448],
                         start=True, stop=False)
        nc.tensor.matmul(ps_Ro[:, :], PMo[0:64, :], Kc[0:64, 448:896],
                         start=False, stop=True)
        nc.scalar.copy(out=R_sb[:, 1, :], in_=ps_Ro[:, :])
        ps_sa = ps.tile([64, 64], F32, tag="ps", name="ps_sa")
        for par in range(2):
            for dy in range(7):
                nc.tensor.matmul(ps_sa[:, :],
                                 Sdy[0:32, 448 * par + 64 * dy:
                                     448 * par + 64 * dy + 64],
                                 R_sb[:, par, bass.ts(dy, 64)],
                                 start=(par == 0 and dy == 0),
                                 stop=(par == 1 and dy == 6))
        nc.scalar.activation(out=sig2d[:, :], in_=ps_sa[:, :], func=AF.Sigmoid)
        nc.sync.dma_start(out=sigrow[:, :].rearrange("p (y x) -> p y x", y=64),
                          in_=sig2d[:, :])

        # =========== final: out = (z*ca)*sigb + (1-w)*x ===========
        for t in range(8):
            ps_bc = ps.tile([128, 512], F32, tag="ps", name="ps_bc")
            nc.tensor.matmul(ps_bc[:, :], _R(C('ones1')[0:1, :]),
                             _R(sigrow[:, bass.ts(t, 512)]), start=True, stop=True)
            sgb = sp.tile([128, 512], BF16, tag="sgb", name="sgb", bufs=2)
            nc.scalar.activation(out=sgb[:, :], in_=ps_bc[:, :], func=AF.Copy)
            vt = sp.tile([128, 2, 512], BF16, tag="vt", name="vt", bufs=2)
            vo = sp.tile([128, 2, 512], BF16, tag="vo", name="vo", bufs=2)
            for ch in range(2):
                nc.vector.tensor_tensor(
                    out=vt[:, ch, :], in0=U_t[:, ch, bass.ts(t, 512)],
                    in1=sgb[:, :], op=ALU.mult)
                nc.vector.tensor_tensor(
                    out=vo[:, ch, :], in0=vt[:, ch, :],
                    in1=xh[:, ch, bass.ts(t, 512)], op=ALU.add)
                nc.sync.dma_start(
                    out=out_d[128 * ch:128 * (ch + 1), bass.ts(t, 512)],
                    in_=vo[:, ch, :])
        stack.close()
        for fr in reversed(_frees):
            fr()
    nc.compile()
    return nc


_CACHE = {}


def kernel(**inputs):
    inp = {k: np.asarray(v) for k, v in inputs.items()}
    f = fold_params(inp)
    key = round(f['w_fuse'], 9)
    if key not in _CACHE:
        _CACHE[key] = build_nc(f['w_fuse'])
    nc = _CACHE[key]

    B = inp['x'].shape[0]
    bf = ml_dtypes.bfloat16
    in_maps = []
    for b in range(B):
        xb = inp['x'][b].reshape(2, 128, N).transpose(1, 0, 2).reshape(128, 2 * N)
        m = {
            'x': np.ascontiguousarray(xb.astype(bf)),
            'x0': np.ascontiguousarray(inp['x0'][b].reshape(128, N).astype(bf)),
            'bfE': f['bfE'],
            'bfL': f['bfL'],
            'f32A': f['f32A'],
            'f32L': f['f32L'],
        }
        in_maps.append(m)

    res = run_bass_kernel_spmd(nc, in_maps, core_ids=list(range(B)))
    out = np.stack([np.asarray(res.results[b]['out']).astype(np.float32)
                    .reshape(256, H, W) for b in range(B)])
    return out
